# revision 8
# baseline (speedup 1.0000x reference)
"""DepAttention kernel for Trainium2 (Bass/Tile), sparse-gather formulation.

score[b,i,j] = (<val[b,i],val[b,j]> + <dep[b,i,j],dep[b,j,i]>) / sqrt(D)
out = exp(score)*adj / (rowsum(exp(score)*adj) + 1e-10)

adj is ~5% dense and out[i,j] = 0 wherever adj[i,j] = 0, so the dep term
(dep is 32MB/core; everything else <1MB) only matters at the ~3.5K nonzero
(i,j) pairs per batch element. The dense baseline streams ~48MB of HBM per
core (~130us); instead we gather just the needed 512B rows of dep (viewed
as [N*N, D]) with the SWDGE dma_gather, ~7MB/core.

Pairs are ordered slot-major (pair k = s*128 + li) so gathered pair k
lands at partition k%128 = li: the gather output is directly row-aligned
[li, s, :]. The B-side (dep[j,i,:]) uses a host-pretransposed copy depT
so both sides share one int16 index list (local flat index li*256+j
< 32768 per 128-row block). Rows with fewer than K edges pad with index
0 (real data, masked later).

Per core, on device:
 1. row-cumsum of adj (DVE scan) -> slot rank of each edge within its row.
 2. local_scatter (128ch) inverts rank->column: jflat16[li,s], used for
    the packed->dense expand indices (idxs2).
 3. spos = 8*slot + (li//16): scatter positions in the 16-partition
    wrapped index layout the gather ucode wants. A DRAM scratch
    round-trip ([128,512] -> [8,16,512] reinterpretation) folds rows
    into 16 partitions; local_scatter (16ch) builds the wrapped index
    list; 7 small copies replicate it to all 8 gpsimd cores.
 4. dma_gather A-rows (dep) and B-rows (depT), chunked; DVE mul+reduce
    over d -> s_dep[li, s].
 5. local_scatter expands packed s_dep into a dense zeroed [128,256]
    tile (f32 as two int16 halves), add the PE val gram, additive-mask
    non-edges, exp (+row-sum accum) on Act, normalize, store.
"""

import numpy as np

import concourse.bacc as bacc
import concourse.tile as tile
import concourse.mybir as mybir
from concourse.bass_utils import run_bass_kernel_spmd

B, N, D = 8, 256, 128
NN = N * N
K = 24          # slots per row; == max nonzeros/row in the data
NCH = 6         # gather chunks per block (num_idxs <= 512 per SWDGE gather)
CH = K // NCH
SCALE = float(1.0 / np.sqrt(np.float32(D)))
BIG = 1024.0    # additive mask: non-edges see exp(-BIG*SCALE) ~ 4e-40
BIAS = float(-(np.float32(BIG) * np.float32(SCALE)))
EPS = 1e-10
F32 = mybir.dt.float32
I16 = mybir.dt.int16
I32 = mybir.dt.int32
AOP = mybir.AluOpType
AXX = mybir.AxisListType.X
EXP = mybir.ActivationFunctionType.Exp
CPY = mybir.ActivationFunctionType.Copy

_NC = None


def host_consts():
    li = np.arange(128, dtype=np.int64)[:, None]
    col = np.arange(2 * K, dtype=np.int64)[None, :]
    flatd16 = (li * 256 + np.arange(256)[None, :] - 32768).astype(np.int16)
    iotaS = ((col % K).astype(np.float32)) * np.ones((128, 1), np.float32)
    negli = (-256 * li).astype(np.float32)
    qli = (li // 16).astype(np.float32)
    p16 = np.arange(16, dtype=np.int64)[:, None]
    qj = np.arange(2048, dtype=np.int64)[None, :]
    flatW = ((16 * (qj // 256) + p16) * 256 + (qj % 256)).astype(np.int16)
    return {
        "flatd16": flatd16,
        "iotaS": np.ascontiguousarray(iotaS),
        "negli": np.ascontiguousarray(negli),
        "qli": np.ascontiguousarray(qli),
        "flatW": np.ascontiguousarray(flatW),
        "biasc": np.full((128, 1), BIAS, dtype=np.float32),
    }


def build_nc(reps=1, dbg=False):
    """reps>1 unrolls the computation (paired-slope timing)."""
    nc = bacc.Bacc("TRN2", target_bir_lowering=False, debug=False, num_devices=8)

    dep = nc.dram_tensor("dep", [NN, D], F32, kind="ExternalInput")
    depT = nc.dram_tensor("depT", [NN, D], F32, kind="ExternalInput")
    valT = nc.dram_tensor("valT", [D, N], F32, kind="ExternalInput")
    adj = nc.dram_tensor("adj", [N, N], F32, kind="ExternalInput")
    flatd16 = nc.dram_tensor("flatd16", [128, 256], I16, kind="ExternalInput")
    iotaS = nc.dram_tensor("iotaS", [128, 2 * K], F32, kind="ExternalInput")
    negli = nc.dram_tensor("negli", [128, 1], F32, kind="ExternalInput")
    qli = nc.dram_tensor("qli", [128, 1], F32, kind="ExternalInput")
    flatWt = nc.dram_tensor("flatW", [16, 2048], I16, kind="ExternalInput")
    biasc = nc.dram_tensor("biasc", [128, 1], F32, kind="ExternalInput")
    out = nc.dram_tensor("out", [N, N], F32, kind="ExternalOutput")
    if dbg:
        d_jflat = nc.dram_tensor("d_jflat", [128, 2 * K], I16, kind="ExternalOutput")
        d_idxw = nc.dram_tensor("d_idxw", [2, 128, 8 * K], I16, kind="ExternalOutput")
        d_sdep = nc.dram_tensor("d_sdep", [128, 2 * K], F32, kind="ExternalOutput")
        d_score = nc.dram_tensor("d_score", [2, 128, N], F32, kind="ExternalOutput")
        d_ev = nc.dram_tensor("d_ev", [2, 128, N], F32, kind="ExternalOutput")

    with tile.TileContext(nc) as tc:
        with (
            tc.tile_pool(name="persist", bufs=1) as pp,
            tc.tile_pool(name="work", bufs=2) as wp,
            tc.tile_pool(name="ga", bufs=4) as gap,
            tc.tile_pool(name="gb", bufs=4) as gbp,
            tc.tile_pool(name="psum", bufs=2, space="PSUM") as psp,
            tc.tile_pool(name="dram", bufs=2, space="DRAM") as dp,
        ):
            vt = pp.tile([D, N], F32, tag="vt")
            adjF = pp.tile([128, 2 * N], F32, tag="adjF")
            c_flat = pp.tile([128, 256], I16, tag="c_flat")
            c_iota = pp.tile([128, 2 * K], F32, tag="c_iota")
            c_negli = pp.tile([128, 1], F32, tag="c_negli")
            c_qli = pp.tile([128, 1], F32, tag="c_qli")
            c_flatW = pp.tile([16, 2048], I16, tag="c_flatW")
            c_bias = pp.tile([128, 1], F32, tag="c_bias")

            nc.sync.dma_start(vt[:], valT[:])
            nc.sync.dma_start(adjF[:, 0:N], adj[0:128, :])
            nc.sync.dma_start(adjF[:, N : 2 * N], adj[128:256, :])
            nc.sync.dma_start(c_flat[:], flatd16[:])
            nc.sync.dma_start(c_iota[:], iotaS[:])
            nc.sync.dma_start(c_negli[:], negli[:])
            nc.sync.dma_start(c_qli[:], qli[:])
            nc.sync.dma_start(c_flatW[:], flatWt[:])
            nc.sync.dma_start(c_bias[:], biasc[:])

            ts = nc.vector.tensor_scalar
            stt = nc.vector.scalar_tensor_tensor
            tt = nc.vector.tensor_tensor

            for _rep in range(reps):
                pos = wp.tile([128, 2 * N], F32, tag="pos")
                slot_t = wp.tile([128, 2 * N], F32, tag="slot_t")
                slot16 = wp.tile([128, 2 * N], I16, tag="slot16")
                sposR = wp.tile([128, 2 * N], F32, tag="sposR")
                sposR16 = wp.tile([128, 2 * N], I16, tag="sposR16")
                scr = dp.tile([8, 16, 512], I16, tag="scr")
                spw = [
                    wp.tile([16, 8, 256], I16, tag=f"spw{i}", name=f"spw{i}")
                    for i in range(2)
                ]
                idxw = [
                    wp.tile([128, 8 * K], I16, tag=f"idxw{i}", name=f"idxw{i}")
                    for i in range(2)
                ]
                jflat16 = wp.tile([128, 2 * K], I16, tag="jflat16")
                bigK = wp.tile([128, 2 * K], F32, tag="bigK")
                j_f = wp.tile([128, 2 * K], F32, tag="j_f")
                validm = wp.tile([128, 2 * K], F32, tag="validm")
                t2 = wp.tile([128, 2 * K], F32, tag="t2")
                tv = wp.tile([128, 2 * K], F32, tag="tv")
                idxs2 = wp.tile([128, 2 * K, 2], I16, tag="idxs2")
                sdep = wp.tile([128, 2 * K], F32, tag="sdep")
                sv = [
                    psp.tile([128, N], F32, tag=f"sv{i}", name=f"sv{i}")
                    for i in range(2)
                ]

                # val gram: sv[b] = valT[:, b].T @ valT  (contraction over d)
                for b2 in range(2):
                    nc.tensor.matmul(
                        sv[b2][:],
                        vt[:, 128 * b2 : 128 * (b2 + 1)],
                        vt[:],
                        start=True,
                        stop=True,
                    )

                # 1. per-row inclusive cumsum of adj, starting at -1
                for b2 in range(2):
                    sl = slice(N * b2, N * (b2 + 1))
                    nc.vector.tensor_tensor_scan(
                        pos[:, sl],
                        adjF[:, sl],
                        adjF[:, sl],
                        -1.0,
                        op0=AOP.add,
                        op1=AOP.bypass,
                    )
                # slot = (pos+1)*adj - 1  (edges: 0-based rank; non-edges: -1)
                stt(slot_t[:], pos[:], 1.0, adjF[:], op0=AOP.add, op1=AOP.mult)
                ts(slot16[:], slot_t[:], -1.0, None, op0=AOP.add)

                # 3. wrapped-layout scatter positions: 8*slot + li//16
                #    (edges >= 0; non-edges -> li//16 - 8 in [-8,-1])
                ts(sposR[:], slot_t[:], 8.0, c_qli[:, 0:1], op0=AOP.mult, op1=AOP.add)
                ts(sposR16[:], sposR[:], -8.0, None, op0=AOP.add)
                # fold rows into 16 partitions via DRAM reinterpretation
                nc.sync.dma_start(scr[:], sposR16[:])
                for b2 in range(2):
                    nc.sync.dma_start(
                        spw[b2][:],
                        scr[:, :, 256 * b2 : 256 * (b2 + 1)].transpose([1, 0, 2]),
                    )
                    nc.gpsimd.local_scatter(
                        idxw[b2][0:16, :],
                        c_flatW[:],
                        spw[b2][:],
                        channels=16,
                        num_elems=8 * K,
                        num_idxs=2048,
                    )
                    for g in range(1, 8):
                        nc.gpsimd.dma_start(
                            idxw[b2][16 * g : 16 * (g + 1), :], idxw[b2][0:16, :]
                        )

                # 2. rank->column per row (for the dense expand)
                for b2 in range(2):
                    nc.gpsimd.local_scatter(
                        jflat16[:, K * b2 : K * (b2 + 1)],
                        c_flat[:],
                        slot16[:, N * b2 : N * (b2 + 1)],
                        channels=128,
                        num_elems=K,
                        num_idxs=256,
                    )
                for b2 in range(2):
                    ksl = slice(K * b2, K * (b2 + 1))
                    ts(
                        bigK[:, ksl],
                        c_iota[:, ksl],
                        pos[:, N * (b2 + 1) - 1 : N * (b2 + 1)],
                        None,
                        op0=AOP.is_gt,
                    )
                # expand indices: (2j, 2j+1) at valid slots, (-2, -1) at pads
                ts(j_f[:], jflat16[:], 32768.0, c_negli[:, 0:1], op0=AOP.add, op1=AOP.add)
                ts(validm[:], bigK[:], -1.0, 1.0, op0=AOP.mult, op1=AOP.add)
                ts(t2[:], j_f[:], 2.0, 2.0, op0=AOP.mult, op1=AOP.add)
                tt(tv[:], t2[:], validm[:], op=AOP.mult)
                ts(idxs2[:, :, 0], tv[:], -2.0, None, op0=AOP.add)
                ts(idxs2[:, :, 1], tv[:], -1.0, None, op0=AOP.add)

                if dbg and _rep == 0:
                    nc.sync.dma_start(d_jflat[:], jflat16[:])
                    for b2 in range(2):
                        nc.sync.dma_start(d_idxw[b2], idxw[b2][:])

                # 4. gather + multiply/reduce, chunked per block
                for b2 in range(2):
                    dsl = slice(32768 * b2, 32768 * (b2 + 1))
                    for c in range(NCH):
                        ksl = slice(K * b2 + c * CH, K * b2 + (c + 1) * CH)
                        isl = slice(8 * c * CH, 8 * (c + 1) * CH)
                        a_t = gap.tile([128, CH, D], F32, tag="a_t", name="a_t")
                        b_t = gbp.tile([128, CH, D], F32, tag="b_t", name="b_t")
                        nc.gpsimd.dma_gather(
                            out_ap=a_t[:],
                            in_ap=dep[dsl, :],
                            idxs_ap=idxw[b2][:, isl],
                            num_idxs=CH * 128,
                            num_idxs_reg=CH * 128,
                            elem_size=D,
                        )
                        nc.gpsimd.dma_gather(
                            out_ap=b_t[:],
                            in_ap=depT[dsl, :],
                            idxs_ap=idxw[b2][:, isl],
                            num_idxs=CH * 128,
                            num_idxs_reg=CH * 128,
                            elem_size=D,
                        )
                        nc.vector.tensor_mul(a_t[:], a_t[:], b_t[:])
                        nc.vector.reduce_sum(sdep[:, ksl], a_t[:], axis=AXX)

                # 5. expand to dense, add gram, mask, exp, normalize, store
                for b2 in range(2):
                    ksl = slice(K * b2, K * (b2 + 1))
                    nsl = slice(N * b2, N * (b2 + 1))
                    dense = wp.tile([128, 2 * N], I16, tag=f"dense{b2}", name=f"dense{b2}")
                    score = wp.tile([128, N], F32, tag=f"score{b2}", name=f"score{b2}")
                    masked = wp.tile([128, N], F32, tag=f"masked{b2}", name=f"masked{b2}")
                    ev = wp.tile([128, N], F32, tag=f"ev{b2}", name=f"ev{b2}")
                    outv = wp.tile([128, N], F32, tag=f"outv{b2}", name=f"outv{b2}")
                    den = wp.tile([128, 1], F32, tag=f"den{b2}", name=f"den{b2}")
                    den2 = wp.tile([128, 1], F32, tag=f"den2{b2}", name=f"den2{b2}")
                    rec = wp.tile([128, 1], F32, tag=f"rec{b2}", name=f"rec{b2}")
                    nc.gpsimd.local_scatter(
                        dense[:],
                        sdep[:, ksl].bitcast(I16),
                        idxs2[:, ksl, :],
                        channels=128,
                        num_elems=2 * N,
                        num_idxs=2 * K,
                    )
                    tt(score[:], dense[:].bitcast(F32), sv[b2][:], op=AOP.add)
                    stt(masked[:], score[:], BIG, adjF[:, nsl], op0=AOP.add, op1=AOP.mult)
                    nc.scalar.activation(
                        ev[:], masked[:], EXP, bias=c_bias[:, 0:1], scale=SCALE,
                        accum_out=den[:],
                    )
                    ts(den2[:], den[:], float(EPS), None, op0=AOP.add)
                    nc.vector.reciprocal(rec[:], den2[:])
                    nc.scalar.activation(outv[:], ev[:], CPY, bias=0.0, scale=rec[:, 0:1])
                    nc.sync.dma_start(out[128 * b2 : 128 * (b2 + 1), :], outv[:])
                    if dbg and _rep == 0:
                        nc.sync.dma_start(d_score[b2], score[:])
                        nc.sync.dma_start(d_ev[b2], ev[:])
                if dbg and _rep == 0:
                    nc.sync.dma_start(d_sdep[:], sdep[:])

    nc.compile()
    return nc


def _get_nc():
    global _NC
    if _NC is None:
        _NC = build_nc()
    return _NC


def make_in_maps(val_out, dep_embed, adj):
    val_out = np.asarray(val_out, dtype=np.float32)
    dep_embed = np.asarray(dep_embed, dtype=np.float32)
    adj = np.asarray(adj, dtype=np.float32)
    assert val_out.shape == (B, N, D)
    assert dep_embed.shape == (B, N, N, D)
    assert adj.shape == (B, N, N)
    cnt = adj.reshape(B, N, N).sum(axis=-1).max()
    assert cnt <= K, f"adjacency row count {cnt} exceeds K={K}"
    consts = host_consts()
    return [
        {
            "dep": np.ascontiguousarray(dep_embed[b].reshape(NN, D)),
            "depT": np.ascontiguousarray(
                dep_embed[b].transpose(1, 0, 2).reshape(NN, D)
            ),
            "valT": np.ascontiguousarray(val_out[b].T),
            "adj": np.ascontiguousarray(adj[b]),
            **consts,
        }
        for b in range(B)
    ]


def kernel(val_out, dep_embed, adj):
    nc = _get_nc()
    in_maps = make_in_maps(val_out, dep_embed, adj)
    res = run_bass_kernel_spmd(nc, in_maps, core_ids=list(range(B)))
    return np.stack([r["out"] for r in res.results])


# revision 9
# speedup vs baseline: 1.7401x; 1.7401x over previous
"""DepAttention kernel for Trainium2 (Bass/Tile), data-parallel over batch.

score[b,i,j] = (<val[b,i],val[b,j]> + <dep[b,i,j],dep[b,j,i]>) / sqrt(D)
out = exp(score)*adj / (rowsum(exp(score)*adj) + 1e-10)

score is symmetric in (i,j) (both terms are), so per core (one batch
element) we compute only the upper block-triangle of the 2x2 grid of
128x128 score blocks -- (0,0), (0,1), (1,1) -- and mirror (0,1) into
(1,0) with a PE transpose. The dep term dominates traffic: each 128-row
x 64-col chunk needs A = dep[iblk, jchunk, :] (contiguous) and
B' = dep[jchunk, iblk, :] with (i,j) swapped (strided AP, 512B runs).
DVE does an in-place multiply then a segmented reduce over d.
"""

import numpy as np

import concourse.bacc as bacc
import concourse.tile as tile
import concourse.mybir as mybir
from concourse.bass_utils import run_bass_kernel_spmd

B, N, D = 8, 256, 128
TJ = 32  # columns per dep chunk
GP_CHUNKS = set()  # chunk indices whose multiply runs on GPSIMD (hurt perf)
SCALE = 1.0 / np.sqrt(np.float32(D))
EPS = 1e-10
F32 = mybir.dt.float32

_NC = None


def build_nc(reps=1, ring_mix=False):
    """reps>1 unrolls the whole computation N times (for timing: the
    wall-clock delta between reps=R and reps=1 isolates device time)."""
    nc = bacc.Bacc("TRN2", target_bir_lowering=False, debug=False, num_devices=8)

    dep = nc.dram_tensor("dep", [N, N, D], F32, kind="ExternalInput")
    valT = nc.dram_tensor("valT", [D, N], F32, kind="ExternalInput")
    adj = nc.dram_tensor("adj", [N, N], F32, kind="ExternalInput")
    ident = nc.dram_tensor("ident", [128, 128], F32, kind="ExternalInput")
    out = nc.dram_tensor("out", [N, N], F32, kind="ExternalOutput")

    with tile.TileContext(nc) as tc:
        with (
            tc.tile_pool(name="a", bufs=5) as a_pool,
            tc.tile_pool(name="b", bufs=5) as b_pool,
            tc.tile_pool(name="persist", bufs=1) as pp,
            tc.tile_pool(name="psum", bufs=1, space="PSUM") as psp,
        ):
            # persistent tiles
            vt = pp.tile([D, N], F32, tag="vt")
            id_t = pp.tile([128, 128], F32, tag="id")
            adj_t = [
                pp.tile([128, N], F32, tag=f"adj{i}", name=f"adj{i}") for i in range(2)
            ]

            nc.gpsimd.dma_start(vt[:], valT[:])
            nc.gpsimd.dma_start(id_t[:], ident[:])
            for i in range(2):
                nc.gpsimd.dma_start(adj_t[i][:], adj[128 * i : 128 * (i + 1), :])

            for _rep in range(reps):
                score = [
                    pp.tile([128, N], F32, tag=f"score{i}", name=f"score{i}", bufs=2)
                    for i in range(2)
                ]
                expv = [
                    pp.tile([128, N], F32, tag=f"expv{i}", name=f"expv{i}", bufs=2)
                    for i in range(2)
                ]
                den = [
                    pp.tile([128, 1], F32, tag=f"den{i}", name=f"den{i}", bufs=2)
                    for i in range(2)
                ]
                rec = [
                    pp.tile([128, 1], F32, tag=f"rec{i}", name=f"rec{i}", bufs=2)
                    for i in range(2)
                ]
                psum_sv = [
                    psp.tile([128, N], F32, tag=f"sv{i}", name=f"sv{i}", bufs=2)
                    for i in range(2)
                ]
                # val part: score_val[I] = valT[:, I*128:+128].T @ valT -> PSUM
                for i in range(2):
                    nc.tensor.matmul(
                        psum_sv[i][:],
                        vt[:, 128 * i : 128 * (i + 1)],
                        vt[:],
                        start=True,
                        stop=True,
                    )

                # dep part: blocks (I,J) with J >= I, chunks of TJ columns.
                # The very first chunk is split into 8-column sub-chunks so
                # the DVE starts ~3us in instead of waiting for a full 2MB
                # load pair (the single-shot ramp); the last chunk is split
                # in half to shorten the epilogue tail.
                nch = 128 // TJ
                work = []
                for (bi, bj) in ((0, 1), (0, 0), (1, 1)):
                    for c in range(nch):
                        work.append((bi, bj, 128 * bj + c * TJ, TJ))
                first = work.pop(0)
                second = work.pop(0)
                work = (
                    [(first[0], first[1], first[2], 4), (first[0], first[1], first[2] + 4, 4)]
                    + [
                        (first[0], first[1], first[2] + 8 + s * 8, 8)
                        for s in range((TJ - 8) // 8)
                    ]
                    + [
                        (second[0], second[1], second[2] + s * 16, 16)
                        for s in range(TJ // 16)
                    ]
                    + work
                )
                last = work.pop()
                work += [
                    (last[0], last[1], last[2] + s * (TJ // 2), TJ // 2)
                    for s in range(2)
                ]
                for idx, (bi, bj, j0, w) in enumerate(work):
                    i0 = 128 * bi
                    a_t = a_pool.tile([128, w, D], F32, name="a_t", tag="a_t")
                    b_t = b_pool.tile([128, w, D], F32, name="b_t", tag="b_t")
                    if ring_mix and idx % 2:
                        a_eng, b_eng = nc.scalar, nc.sync
                    else:
                        a_eng, b_eng = nc.sync, nc.scalar
                    a_eng.dma_start(a_t[:], dep[i0 : i0 + 128, j0 : j0 + w, :])
                    b_eng.dma_start(
                        b_t[:],
                        dep[j0 : j0 + w, i0 : i0 + 128, :].transpose([1, 0, 2]),
                    )
                    nc.vector.tensor_mul(a_t[:], a_t[:], b_t[:])
                    nc.vector.reduce_sum(
                        score[bi][:, j0 : j0 + w],
                        a_t[:],
                        axis=mybir.AxisListType.X,
                    )

                # mirror dep block (0,1) -> (1,0): PE transpose (reads the
                # pure dep part of score0 before val is added in-place below)
                psum_t = psp.tile([128, 128], F32, tag="pt", name="pt", bufs=2)
                nc.tensor.transpose(psum_t[:], score[0][:, 128:256], id_t[:])
                nc.scalar.copy(score[1][:, 0:128], psum_t[:])

                # epilogue. Row 0 whole; row 1 split at col 224 so the
                # head processes while the last (1,1) chunks still compute:
                # den1 = sum of two partial row-sums.
                den1b = pp.tile([128, 1], F32, tag="den1b", name="den1b", bufs=2)
                for i in range(2):
                    parts = [(0, 256)] if i == 0 else [(0, 224), (224, 256)]
                    for lo, hi in parts:
                        nc.vector.tensor_add(
                            score[i][:, lo:hi], score[i][:, lo:hi], psum_sv[i][:, lo:hi]
                        )
                        nc.scalar.activation(
                            expv[i][:, lo:hi],
                            score[i][:, lo:hi],
                            mybir.ActivationFunctionType.Exp,
                            scale=float(SCALE),
                        )
                        nc.vector.tensor_mul(
                            expv[i][:, lo:hi], expv[i][:, lo:hi], adj_t[i][:, lo:hi]
                        )
                        tgt = den[i] if lo == 0 else den1b
                        nc.vector.reduce_sum(
                            tgt[:], expv[i][:, lo:hi], axis=mybir.AxisListType.X
                        )
                    if i == 1:
                        nc.vector.tensor_add(den[i][:], den[i][:], den1b[:])
                    nc.vector.tensor_scalar_add(den[i][:], den[i][:], float(EPS))
                    nc.vector.reciprocal(rec[i][:], den[i][:])
                    nc.vector.tensor_scalar_mul(expv[i][:], expv[i][:], rec[i][:, 0:1])
                    nc.sync.dma_start(out[128 * i : 128 * (i + 1), :], expv[i][:])

    nc.compile()
    return nc


def _get_nc():
    global _NC
    if _NC is None:
        _NC = build_nc()
    return _NC


def kernel(val_out, dep_embed, adj):
    val_out = np.asarray(val_out, dtype=np.float32)
    dep_embed = np.asarray(dep_embed, dtype=np.float32)
    adj = np.asarray(adj, dtype=np.float32)
    assert val_out.shape == (B, N, D)
    assert dep_embed.shape == (B, N, N, D)
    assert adj.shape == (B, N, N)

    nc = _get_nc()
    ident = np.eye(128, dtype=np.float32)
    in_maps = [
        {
            "dep": np.ascontiguousarray(dep_embed[b]),
            "valT": np.ascontiguousarray(val_out[b].T),
            "adj": np.ascontiguousarray(adj[b]),
            "ident": ident,
        }
        for b in range(B)
    ]
    res = run_bass_kernel_spmd(nc, in_maps, core_ids=list(range(B)))
    return np.stack([r["out"] for r in res.results])

